# revision 5
# baseline (speedup 1.0000x reference)
# Trainium2 Bass kernel for EvidenceRetriever (cosine-sim retrieval + top-8).
#
# score[t, s] = <t_hat, s_hat> + 0.1 * importance[s]   (t_hat/s_hat L2-normalized)
# outputs: top-8 indices (int32), top-8 scores (f32, desc), softmax over the 8.
#
# Sharding: data-parallel over target rows - 8 cores x 2048 rows each;
# source_edge_feats + importance replicated. No cross-core communication.
#
# Per-core design (fp8 DoubleRow + fp16 index-encode + exact rescore):
#   prep:    normalize targets/sources to fp16 (ACT scale-copy), DMA-XBAR
#            transpose fp16 128x128 blocks into k-tile-plane slabs, gpsimd
#            converts slabs to fp8 for the PE. Importance bias is split
#            hi/lo into two fp8 rows (4e-4 abs residual).
#   phase A: per source chunk: 2 DoubleRow fp8 matmuls per 512-seg (K=512
#            as 2 plane-pairs) + 1 DoubleRow rank-2 bias matmul -> PSUM
#            scores. ACT copies PSUM to the fp16 high halves of a u32
#            "encode" tile whose low u16s hold global source ids (gpsimd
#            iota). One DVE max8 over the f32-viewed encode tile then
#            yields top-8 (value, id) pairs per row with NO max_index pass.
#   stage 2: top-16 encoded candidates via max8 + match_replace + max8
#            (encodings are globally unique, so duplicates are impossible);
#            ids = low 16 bits.
#   phase B: gather candidates' normalized fp16 rows + importance by id,
#            fused DVE dot vs fp16 targets, + bias -> near-exact scores;
#            final top-8 by exact score, ids via unique positions, softmax.
import os
from contextlib import ExitStack

import numpy as np

import concourse.bass as bass
import concourse.tile as tile
from concourse import bacc, mybir
from concourse.bass_utils import run_bass_kernel_spmd

F32 = mybir.dt.float32
F16 = mybir.dt.float16
F8 = mybir.dt.float8e4
U32 = mybir.dt.uint32
I32 = mybir.dt.int32
AF = mybir.ActivationFunctionType
ALU = mybir.AluOpType
PM = mybir.MatmulPerfMode

N_CORES = 8
E_T, E_S, FDIM = 16384, 32768, 512
T_LOCAL = E_T // N_CORES
K = 8
NCAND = 16            # candidates rescored near-exactly
W_IMPORTANCE = 0.1
CHUNK = 2048          # source cols per PSUM tile (4 banks)
SUPER = 2             # chunks per max8 scan
N_CH = E_S // CHUNK   # 16
N_GRP = N_CH // SUPER
N_TT = T_LOCAL // 128  # 16
N_KT = FDIM // 128     # 4
N_SEG = CHUNK // 512   # 4
CW = N_GRP * K         # compact candidate width per row

LAST_RESULTS = None


def build_program(repeat=1):
    nc = bacc.Bacc(None, target_bir_lowering=False, debug=False)
    t_in = nc.dram_tensor("t", [T_LOCAL, FDIM], F32, kind="ExternalInput")
    s_in = nc.dram_tensor("s", [E_S, FDIM], F32, kind="ExternalInput")
    impt_in = nc.dram_tensor("impt", [E_S, 1], F32, kind="ExternalInput")
    idx_out = nc.dram_tensor("idx", [T_LOCAL, K], I32, kind="ExternalOutput")
    score_out = nc.dram_tensor("score", [T_LOCAL, K], F32, kind="ExternalOutput")
    alpha_out = nc.dram_tensor("alpha", [T_LOCAL, K], F32, kind="ExternalOutput")
    # scratch DRAM
    s16_dram = nc.dram_tensor("s16", [E_S, FDIM], F16, kind="Internal")
    hi_dram = nc.dram_tensor("bhi", [128, E_S // 128], F8, kind="Internal")
    lo_dram = nc.dram_tensor("blo", [128, E_S // 128], F8, kind="Internal")

    with tile.TileContext(nc) as tc:
        with ExitStack() as ctx:
            const = ctx.enter_context(tc.tile_pool(name="const", bufs=1))
            prep = ctx.enter_context(tc.tile_pool(name="prep", bufs=6))
            st16 = ctx.enter_context(tc.tile_pool(name="st16", bufs=2))
            st8 = ctx.enter_context(tc.tile_pool(name="st8", bufs=2))
            biasp = ctx.enter_context(tc.tile_pool(name="biasp", bufs=3))
            mm_psum = ctx.enter_context(
                tc.tile_pool(name="mm", bufs=2, space="PSUM"))
            fin = ctx.enter_context(tc.tile_pool(name="fin", bufs=3))
            gat = ctx.enter_context(tc.tile_pool(name="gat", bufs=4))

            ones8 = const.tile([1, 2, 128], F8)
            nc.vector.memset(ones8[:], 1.0)
            iota_nc_i = const.tile([128, NCAND], I32)
            nc.gpsimd.iota(iota_nc_i[:], pattern=[[1, NCAND]], base=0,
                           channel_multiplier=0)
            iota_nc = const.tile([128, NCAND], F32)
            nc.vector.tensor_copy(iota_nc[:], iota_nc_i[:])

            # residents
            t16 = [const.tile([128, FDIM], F16, name=f"t16_{i}")
                   for i in range(N_TT)]
            tT8 = const.tile([128, N_KT, T_LOCAL], F8)
            cvals = [const.tile([128, CW], F32, name=f"cv{i}")
                     for i in range(N_TT)]
            encs = [const.tile([128, SUPER * CHUNK], U32, name=f"enc{i}")
                    for i in range(2)]

            def normalize16(dst16, src_rows):
                """DMA 128 rows f32, L2-normalize into dst16 (fp16)."""
                raw = prep.tile([128, FDIM], F32, tag="raw")
                nc.sync.dma_start(raw[:], src_rows)
                sq = prep.tile([128, FDIM], F32, tag="sq")
                ss = prep.tile([128, 1], F32, tag="ss")
                nc.scalar.activation(sq[:], raw[:], AF.Square, accum_out=ss[:])
                nrm = prep.tile([128, 1], F32, tag="nrm")
                nc.scalar.sqrt(nrm[:], ss[:])
                inv = prep.tile([128, 1], F32, tag="inv")
                nc.vector.reciprocal(inv[:], nrm[:])
                nc.scalar.activation(dst16[:], raw[:], AF.Copy, scale=inv[:])

            # ---- target prep ----
            ttp = ctx.enter_context(tc.tile_pool(name="ttp", bufs=1))
            tT16 = ttp.tile([128, N_KT, T_LOCAL], F16, tag="tT16")
            for tt in range(N_TT):
                normalize16(t16[tt], t_in.ap()[tt * 128:(tt + 1) * 128, :])
                for kt in range(N_KT):
                    nc.sync.dma_start_transpose(
                        tT16[:, kt, tt * 128:(tt + 1) * 128],
                        t16[tt][:, kt * 128:(kt + 1) * 128])
            nc.gpsimd.tensor_copy(tT8[:], tT16[:])

            # ---- bias hi/lo tables ----
            impv = impt_in.ap().rearrange("(p c) one -> p (c one)", p=128)
            imp_sb = const.tile([128, E_S // 128], F32, name="impsb")
            nc.sync.dma_start(imp_sb[:], impv)
            hi8 = const.tile([128, E_S // 128], F8, name="hi8")
            nc.scalar.activation(hi8[:], imp_sb[:], AF.Copy,
                                 scale=W_IMPORTANCE)
            r32 = const.tile([128, E_S // 128], F32, name="r32")
            nc.vector.scalar_tensor_tensor(
                r32[:], imp_sb[:], W_IMPORTANCE, hi8[:],
                op0=ALU.mult, op1=ALU.subtract)
            lo8 = const.tile([128, E_S // 128], F8, name="lo8")
            nc.vector.tensor_copy(lo8[:], r32[:])
            nc.sync.dma_start(hi_dram.ap(), hi8[:])
            nc.sync.dma_start(lo_dram.ap(), lo8[:])
            hi_rows = hi_dram.ap().rearrange("(a b) c -> a (b c)",
                                             b=128 // N_CH)
            lo_rows = lo_dram.ap().rearrange("(a b) c -> a (b c)",
                                             b=128 // N_CH)

            def prep_chunk(sc, sT16c):
                """normalize+transpose 16 source tiles of chunk sc into
                sT16c [128, N_KT, CHUNK]."""
                for r in range(CHUNK // 128):
                    row0 = sc * CHUNK + r * 128
                    s16d = prep.tile([128, FDIM], F16, tag="s16d")
                    normalize16(s16d, s_in.ap()[row0:row0 + 128, :])
                    nc.sync.dma_start(
                        s16_dram.ap()[row0:row0 + 128, :], s16d[:])
                    for kt in range(N_KT):
                        nc.sync.dma_start_transpose(
                            sT16c[:, kt, r * 128:(r + 1) * 128],
                            s16d[:, kt * 128:(kt + 1) * 128])

            def finish_tile(tt):
                """stage 2 + phase B + outputs for one target tile."""
                c16 = fin.tile([128, NCAND], F32, tag="c16")
                nc.vector.max(c16[:, 0:8], cvals[tt][:])
                scratch = fin.tile([128, CW], F32, tag="scratch")
                nc.vector.match_replace(scratch[:], c16[:, 0:8],
                                        cvals[tt][:], -1e30)
                nc.vector.max(c16[:, 8:16], scratch[:])
                ids_i = fin.tile([128, NCAND], U32, tag="idsi")
                nc.vector.tensor_scalar(ids_i[:], c16[:].bitcast(U32),
                                        65535, None, op0=ALU.bitwise_and)
                idsf = fin.tile([128, NCAND], F32, tag="idsf")
                nc.vector.tensor_copy(idsf[:], ids_i[:])

                exact = fin.tile([128, NCAND], F32, tag="exact")
                for m in range(NCAND):
                    g16 = gat.tile([128, FDIM], F16, tag="g16")
                    nc.gpsimd.indirect_dma_start(
                        out=g16[:], out_offset=None,
                        in_=s16_dram.ap(),
                        in_offset=bass.IndirectOffsetOnAxis(
                            ap=ids_i[:, m:m + 1], axis=0))
                    gb = gat.tile([128, 1], F32, tag="gb")
                    nc.gpsimd.indirect_dma_start(
                        out=gb[:], out_offset=None,
                        in_=impt_in.ap(),
                        in_offset=bass.IndirectOffsetOnAxis(
                            ap=ids_i[:, m:m + 1], axis=0))
                    junk = gat.tile([128, FDIM], F32, tag="junk")
                    dotc = gat.tile([128, 1], F32, tag="dotc")
                    nc.vector.scalar_tensor_tensor(
                        junk[:], g16[:], 1.0, t16[tt][:],
                        op0=ALU.bypass, op1=ALU.mult, accum_out=dotc[:])
                    nc.vector.scalar_tensor_tensor(
                        exact[:, m:m + 1], gb[:], W_IMPORTANCE, dotc[:],
                        op0=ALU.mult, op1=ALU.add)

                fvals = fin.tile([128, K], F32, tag="fvals")
                nc.vector.max(fvals[:], exact[:])
                fpos = fin.tile([128, K], U32, tag="fpos")
                nc.vector.max_index(fpos[:], fvals[:], exact[:])
                fposf = fin.tile([128, K], F32, tag="fposf")
                nc.vector.tensor_copy(fposf[:], fpos[:])
                gidx_f = fin.tile([128, K], F32, tag="gixf")
                junk2 = fin.tile([128, NCAND], F32, tag="junk2")
                for k in range(K):
                    nc.vector.scalar_tensor_tensor(
                        junk2[:], iota_nc[:], fposf[:, k:k + 1], idsf[:],
                        op0=ALU.is_equal, op1=ALU.mult,
                        accum_out=gidx_f[:, k:k + 1])
                gidx_i = fin.tile([128, K], I32, tag="gixi")
                nc.vector.tensor_copy(gidx_i[:], gidx_f[:])
                e = fin.tile([128, K], F32, tag="e")
                sume = fin.tile([128, 1], F32, tag="sume")
                nc.scalar.activation(e[:], fvals[:], AF.Exp,
                                     accum_out=sume[:])
                rse = fin.tile([128, 1], F32, tag="rse")
                nc.vector.reciprocal(rse[:], sume[:])
                alpha_t = fin.tile([128, K], F32, tag="al")
                nc.vector.tensor_scalar_mul(alpha_t[:], e[:], rse[:])

                rows = slice(tt * 128, (tt + 1) * 128)
                nc.sync.dma_start(idx_out.ap()[rows, :], gidx_i[:])
                nc.sync.dma_start(score_out.ap()[rows, :], fvals[:])
                nc.sync.dma_start(alpha_out.ap()[rows, :], alpha_t[:])

            # ---- main loop ----
            for rep in range(repeat):
                for sg in range(N_GRP):
                    sT8g = st8.tile([128, N_KT, SUPER * CHUNK], F8,
                                    tag="sT8g")
                    bias_ts = []
                    for j in range(SUPER):
                        sc = sg * SUPER + j
                        sT16c = st16.tile([128, N_KT, CHUNK], F16,
                                          tag="sT16c")
                        prep_chunk(sc, sT16c)
                        nc.gpsimd.tensor_copy(
                            sT8g[:, :, j * CHUNK:(j + 1) * CHUNK], sT16c[:])
                        bt = biasp.tile([1, 2, CHUNK], F8, tag="bt")
                        nc.sync.dma_start(bt[0:1, 0, :], hi_rows[sc:sc + 1, :])
                        nc.sync.dma_start(bt[0:1, 1, :], lo_rows[sc:sc + 1, :])
                        bias_ts.append(bt)
                    for b in range(2):
                        nc.gpsimd.iota(encs[b][:],
                                       pattern=[[1, SUPER * CHUNK]],
                                       base=sg * SUPER * CHUNK,
                                       channel_multiplier=0)
                    for tt in range(N_TT):
                        enc = encs[tt % 2]
                        enc16 = enc[:].bitcast(F16).rearrange(
                            "p (n two) -> p n two", two=2)
                        for j in range(SUPER):
                            ps = mm_psum.tile([128, CHUNK], F32, tag="ps")
                            for n in range(N_SEG):
                                seg = ps[:, n * 512:(n + 1) * 512]
                                mseg = slice(j * CHUNK + n * 512,
                                             j * CHUNK + (n + 1) * 512)
                                nc.tensor.matmul(
                                    seg,
                                    tT8[:, 0:2, tt * 128:(tt + 1) * 128],
                                    sT8g[:, 0:2, mseg],
                                    start=True, stop=False,
                                    perf_mode=PM.DoubleRow)
                                nc.tensor.matmul(
                                    seg,
                                    tT8[:, 2:4, tt * 128:(tt + 1) * 128],
                                    sT8g[:, 2:4, mseg],
                                    start=False, stop=False,
                                    perf_mode=PM.DoubleRow)
                                nc.tensor.matmul(
                                    seg, ones8[:],
                                    bias_ts[j][:, :, n * 512:(n + 1) * 512],
                                    start=False, stop=True,
                                    perf_mode=PM.DoubleRow)
                            nc.scalar.activation(
                                enc16[:, j * CHUNK:(j + 1) * CHUNK, 1],
                                ps[:], AF.Copy)
                        nc.vector.max(cvals[tt][:, sg * K:(sg + 1) * K],
                                      enc[:].bitcast(F32))
                        if sg == N_GRP - 1:
                            finish_tile(tt)

    nc.compile()
    return nc


_COMPILED = None


def _get_compiled():
    global _COMPILED
    if _COMPILED is None:
        _COMPILED = build_program()
    return _COMPILED


def make_in_maps(t, s, imp):
    t = np.ascontiguousarray(np.asarray(t, dtype=np.float32))
    s = np.ascontiguousarray(np.asarray(s, dtype=np.float32))
    impt = np.ascontiguousarray(
        np.asarray(imp, dtype=np.float32).reshape(-1, 1))
    assert t.shape == (E_T, FDIM) and s.shape == (E_S, FDIM)
    return [
        {"t": t[i * T_LOCAL:(i + 1) * T_LOCAL], "s": s, "impt": impt}
        for i in range(N_CORES)
    ]


def kernel(target_edge_feats, source_edge_feats, source_importance,
           topk=8, chunk_size=4096):
    global LAST_RESULTS
    assert int(topk) == K
    nc = _get_compiled()
    in_maps = make_in_maps(target_edge_feats, source_edge_feats,
                           source_importance)
    res = run_bass_kernel_spmd(
        nc, in_maps, list(range(N_CORES)),
        trace=bool(os.environ.get("BASS_TRACE")))
    LAST_RESULTS = res
    idx = np.concatenate(
        [res.results[i]["idx"] for i in range(N_CORES)], axis=0)
    score = np.concatenate(
        [res.results[i]["score"] for i in range(N_CORES)], axis=0)
    alpha = np.concatenate(
        [res.results[i]["alpha"] for i in range(N_CORES)], axis=0)
    return idx.astype(np.int32), score.astype(np.float32), alpha.astype(np.float32)
